# revision 1
# baseline (speedup 1.0000x reference)
"""Haar wavelet (2x2 stride-2, per-channel) Trainium2 Bass kernel.

Full input x: (8, 64, 512, 512) f32 -> full output (8, 256, 256, 256) f32.
Sharding: pure data parallel over batch -- core i processes x[i].

Per-core kernel layout (C=64 channels, H=W=512):
  - Block = KC channels x 128 output rows. One load DMA per block
    (128, KC*1024): partition p holds input rows (2*(i0+p), 2*(i0+p)+1)
    -- 4KB contiguous runs in DRAM -- for KC channels.
  - Halve in place (DVE tensor_scalar 2x), then
  - Vertical butterfly (DVE): s = top + bot ; d = bot - top
  - Horizontal butterfly (DVE, stride-2 reads):
      ll = s_e + s_o ; lh = d_e + d_o ; hl = s_o - s_e ; hh = d_o - d_e
  - One store DMA per block: the block's 4*KC output channels are
    contiguous in DRAM (channel layout [c*(ll,lh,hl,hh)]).
Engine roles: ACT = load ring, SP = store ring, DVE = all compute.
All compute on one engine keeps every instruction at <=2 sync waits
(the walrus codegen limit). Emission is software-pipelined (stage2 of
block i-1 after stage1 of block i) so no engine idles.
"""

import sys

if "/opt/trn_rl_repo" not in sys.path:
    sys.path.insert(0, "/opt/trn_rl_repo")

from contextlib import ExitStack

import numpy as np

import concourse.bass as bass
import concourse.tile as tile
from concourse import bacc
from concourse import mybir
from concourse.bass_utils import run_bass_kernel_spmd

N_CORES = 8
C, H, W = 64, 512, 512
F32 = mybir.dt.float32
ADD = mybir.AluOpType.add
SUB = mybir.AluOpType.subtract

_CACHED = {}


def _build(C=C, H=H, W=W, KC=4, P=128):
    HO, WO = H // 2, W // 2
    N_HB = HO // P
    nc = bacc.Bacc("TRN2", target_bir_lowering=False, debug=False)
    x = nc.dram_tensor("x", [C, H, W], F32, kind="ExternalInput").ap()
    out = nc.dram_tensor("out", [4 * C, HO, WO], F32, kind="ExternalOutput").ap()

    blocks = [(cg * KC, hb * P) for cg in range(C // KC) for hb in range(N_HB)]

    with tile.TileContext(nc) as tc, ExitStack() as ctx:
        xpool = ctx.enter_context(tc.tile_pool(name="xp", bufs=5))
        mpool = ctx.enter_context(tc.tile_pool(name="mid", bufs=2))
        rpool = ctx.enter_context(tc.tile_pool(name="raw", bufs=4))

        pending = None  # (s_t, d_t, c0, i0) awaiting stage2 + halve + store

        def stage2_and_store(s_t, d_t, c0, i0):
            s2 = s_t[:].rearrange("p (k j t) -> p k j t", k=KC, t=2)
            d2 = d_t[:].rearrange("p (k j t) -> p k j t", k=KC, t=2)
            s_e, s_o = s2[:, :, :, 0], s2[:, :, :, 1]
            d_e, d_o = d2[:, :, :, 0], d2[:, :, :, 1]

            rt = rpool.tile([P, KC * 4 * WO], F32)
            r4 = rt[:].rearrange("p (k q j) -> p k q j", k=KC, q=4)
            nc.vector.tensor_tensor(r4[:, :, 0, :], s_e, s_o, ADD)  # ll
            nc.vector.tensor_tensor(r4[:, :, 1, :], d_e, d_o, ADD)  # lh
            nc.vector.tensor_tensor(r4[:, :, 2, :], s_o, s_e, SUB)  # hl
            nc.vector.tensor_tensor(r4[:, :, 3, :], d_o, d_e, SUB)  # hh

            # One store DMA: the block's 4*KC output channels are
            # contiguous in DRAM.
            dst = out[4 * c0 : 4 * (c0 + KC), i0 : i0 + P, :].transpose([1, 0, 2])
            nc.sync.dma_start(dst, rt[:].rearrange("p (c j) -> p c j", j=WO))

        for c0, i0 in blocks:
            # ---- load: (128, KC, 1024); p holds rows 2*(i0+p), 2*(i0+p)+1
            xt = xpool.tile([P, KC * 2 * W], F32)
            src = x[c0 : c0 + KC, 2 * i0 : 2 * i0 + 2 * P, :].rearrange(
                "k (p t) w -> p k (t w)", t=2
            )
            nc.scalar.dma_start(xt[:].rearrange("p (k f) -> p k f", k=KC), src)

            # ---- halve in place (DVE tensor_scalar, 2x mode)
            nc.vector.tensor_scalar_mul(xt[:], xt[:], 0.5)

            x4 = xt[:].rearrange("p (k t w) -> p k t w", k=KC, t=2)
            top, bot = x4[:, :, 0, :], x4[:, :, 1, :]

            # ---- vertical butterfly (DVE)
            s_t = mpool.tile([P, KC * W], F32)
            d_t = mpool.tile([P, KC * W], F32)
            sv = s_t[:].rearrange("p (k w) -> p k w", k=KC)
            dv = d_t[:].rearrange("p (k w) -> p k w", k=KC)
            nc.vector.tensor_tensor(sv, top, bot, ADD)
            nc.vector.tensor_tensor(dv, bot, top, SUB)

            # ---- previous block's stage2 + halve + store (software pipelining)
            if pending is not None:
                stage2_and_store(*pending)
            pending = (s_t, d_t, c0, i0)

        stage2_and_store(*pending)
    nc.compile()
    return nc


def _get_nc():
    if "nc" not in _CACHED:
        _CACHED["nc"] = _build()
    return _CACHED["nc"]


def _run(x, **kwargs):
    x = np.ascontiguousarray(np.asarray(x), dtype=np.float32)
    assert x.shape == (N_CORES, C, H, W), x.shape
    nc = _get_nc()
    in_maps = [{"x": np.ascontiguousarray(x[i])} for i in range(N_CORES)]
    res = run_bass_kernel_spmd(nc, in_maps, core_ids=list(range(N_CORES)), **kwargs)
    out = np.stack([res.results[i]["out"] for i in range(N_CORES)], axis=0)
    return out, res


def kernel(x):
    return _run(x)[0]



# revision 4
# speedup vs baseline: 1.0683x; 1.0683x over previous
"""Haar wavelet (2x2 stride-2, per-channel) Trainium2 Bass kernel.

Full input x: (8, 64, 512, 512) f32 -> full output (8, 256, 256, 256) f32.
Sharding: pure data parallel over batch -- core i processes x[i].

Per-core layout (C=64 channels, H=W=512), v2:
  - Block = KC=2 channels x full height. Partition p = k*64 + b holds
    input rows 8b..8b+7 of channel c0+k: one 16 KB contiguous DRAM run
    per partition per load (4 KB DMA packets).
  - ACT engine halves the tile in place (activation Copy, scale=0.5),
    freeing the DVE of one full pass (fp32 tensor_scalar is the same
    element count as a butterfly stage).
  - DVE vertical butterfly (2 ops, FD 2048): s = top+bot, d = bot-top,
    written interleaved into one mid tile m = (v, a, w).
  - DVE horizontal butterfly (2 ops, FD 2048): the (s,d) interleave
    makes (ll,lh) = even+odd and (hl,hh) = odd-even each a single
    tensor_tensor over v in {s,d}.
  - Store: partition p holds 4 output rows x 4 subbands of one channel:
    4 runs of 4 KB contiguous DRAM each (vs 16 runs of 1 KB in v1).
Engine roles: ACT = load ring + halve, SP = store ring, DVE = butterflies.
Emission prefetches loads 2 blocks ahead so the load trigger is never
queued behind a stalled halve on the ACT queue.
Roofline: 134 MB HBM traffic / ~358 GB/s per core = ~375 us; DVE is
4 ops x (2048+151) cyc x 32 blocks / 0.96 GHz = ~293 us (hidden).
"""

import sys

if "/opt/trn_rl_repo" not in sys.path:
    sys.path.insert(0, "/opt/trn_rl_repo")

from contextlib import ExitStack

import numpy as np

import concourse.bass as bass
import concourse.tile as tile
from concourse import bacc
from concourse import mybir
from concourse.bass_utils import run_bass_kernel_spmd

N_CORES = 8
C, H, W = 64, 512, 512
F32 = mybir.dt.float32
ADD = mybir.AluOpType.add
SUB = mybir.AluOpType.subtract

_CACHED = {}


def _build(C=C, H=H, W=W, KC=2, R=8):
    HO, WO = H // 2, W // 2
    A = R // 2               # output rows per partition
    PB = H // R              # partitions per channel (64)
    assert KC * PB == 128
    n_blocks = C // KC
    FD = R * W               # free-dim elems per partition (4096)

    nc = bacc.Bacc("TRN2", target_bir_lowering=False, debug=False)
    x = nc.dram_tensor("x", [C, H, W], F32, kind="ExternalInput").ap()
    out = nc.dram_tensor("out", [4 * C, HO, WO], F32, kind="ExternalOutput").ap()

    with tile.TileContext(nc) as tc, ExitStack() as ctx:
        xpool = ctx.enter_context(tc.tile_pool(name="xp", bufs=3))
        mpool = ctx.enter_context(tc.tile_pool(name="mp", bufs=2))
        rpool = ctx.enter_context(tc.tile_pool(name="rp", bufs=3))

        xts = {}

        def emit_load(i):
            c0 = i * KC
            xt = xpool.tile([128, FD], F32)
            src = x[c0 : c0 + KC].rearrange("k (b f) w -> (k b) f w", f=R)
            nc.scalar.dma_start(xt[:].rearrange("p (f w) -> p f w", w=W), src)
            xts[i] = xt

        def emit_compute_store(i):
            c0 = i * KC
            xt = xts.pop(i)

            # ---- halve in place on ACT (activation Copy, scale 0.5)
            nc.scalar.mul(xt[:], xt[:], 0.5)

            x4 = xt[:].rearrange("p (a t w) -> p a t w", t=2, w=W)
            top, bot = x4[:, :, 0, :], x4[:, :, 1, :]

            # ---- vertical butterfly (DVE), s/d interleaved in one tile
            m_t = mpool.tile([128, 2 * A * W], F32)
            mv = m_t[:].rearrange("p (v a w) -> p v a w", v=2, a=A)
            nc.vector.tensor_tensor(mv[:, 0], top, bot, ADD)   # s
            nc.vector.tensor_tensor(mv[:, 1], bot, top, SUB)   # d

            # ---- horizontal butterfly (DVE), 2 fused ops
            m5 = m_t[:].rearrange("p (v a j t) -> p v a j t", v=2, a=A, t=2)
            ev, od = m5[:, :, :, :, 0], m5[:, :, :, :, 1]
            rt = rpool.tile([128, 4 * A * WO], F32)
            r4 = rt[:].rearrange("p (u a j) -> p u a j", u=4, a=A)
            nc.vector.tensor_tensor(r4[:, 0:2], ev, od, ADD)   # ll, lh
            nc.vector.tensor_tensor(r4[:, 2:4], od, ev, SUB)   # hl, hh

            # ---- store: 4 runs of A*WO*4 = 4 KB contiguous per partition.
            # DMA APs are capped at 3 dims, so one DMA per channel:
            # dst (b, q, r*w), src = partition slice (b, q, r*w).
            for k in range(KC):
                ck = c0 + k
                dst = out[4 * ck : 4 * ck + 4].rearrange(
                    "q (b r) w -> b q (r w)", r=A
                )
                src = rt[k * PB : (k + 1) * PB].rearrange(
                    "b (q f) -> b q f", q=4
                )
                nc.sync.dma_start(dst, src)

        emit_load(0)
        emit_load(1)
        for i in range(n_blocks):
            if i + 2 < n_blocks:
                emit_load(i + 2)
            emit_compute_store(i)
    nc.compile()
    return nc


def _get_nc():
    if "nc" not in _CACHED:
        _CACHED["nc"] = _build()
    return _CACHED["nc"]


def _run(x, **kwargs):
    x = np.ascontiguousarray(np.asarray(x), dtype=np.float32)
    assert x.shape == (N_CORES, C, H, W), x.shape
    nc = _get_nc()
    in_maps = [{"x": np.ascontiguousarray(x[i])} for i in range(N_CORES)]
    res = run_bass_kernel_spmd(nc, in_maps, core_ids=list(range(N_CORES)), **kwargs)
    out = np.stack([res.results[i]["out"] for i in range(N_CORES)], axis=0)
    return out, res


def kernel(x):
    return _run(x)[0]


# revision 6
# speedup vs baseline: 1.0947x; 1.0247x over previous
"""Haar wavelet (2x2 stride-2, per-channel) Trainium2 Bass kernel.

Full input x: (8, 64, 512, 512) f32 -> full output (8, 256, 256, 256) f32.
Sharding: pure data parallel over batch -- core i processes x[i].

Per-core layout (C=64 channels, H=W=512), v2:
  - Block = KC=2 channels x full height. Partition p = k*64 + b holds
    input rows 8b..8b+7 of channel c0+k: one 16 KB contiguous DRAM run
    per partition per load (4 KB DMA packets).
  - ACT engine halves the tile in place (activation Copy, scale=0.5),
    freeing the DVE of one full pass (fp32 tensor_scalar is the same
    element count as a butterfly stage).
  - DVE vertical butterfly (2 ops, FD 2048): s = top+bot, d = bot-top,
    written interleaved into one mid tile m = (v, a, w).
  - DVE horizontal butterfly (2 ops, FD 2048): the (s,d) interleave
    makes (ll,lh) = even+odd and (hl,hh) = odd-even each a single
    tensor_tensor over v in {s,d}.
  - Store: partition p holds 4 output rows x 4 subbands of one channel:
    4 runs of 4 KB contiguous DRAM each (vs 16 runs of 1 KB in v1).
Engine roles: ACT = load ring + halve, SP = store ring, DVE = butterflies.
Emission prefetches loads 2 blocks ahead so the load trigger is never
queued behind a stalled halve on the ACT queue.
Roofline: 134 MB HBM traffic / ~358 GB/s per core = ~375 us; DVE is
4 ops x (2048+151) cyc x 32 blocks / 0.96 GHz = ~293 us (hidden).
"""

import sys

if "/opt/trn_rl_repo" not in sys.path:
    sys.path.insert(0, "/opt/trn_rl_repo")

from contextlib import ExitStack

import numpy as np

import concourse.bass as bass
import concourse.tile as tile
from concourse import bacc
from concourse import mybir
from concourse.bass_utils import run_bass_kernel_spmd

N_CORES = 8
C, H, W = 64, 512, 512
F32 = mybir.dt.float32
ADD = mybir.AluOpType.add
SUB = mybir.AluOpType.subtract

_CACHED = {}


def _build(C=C, H=H, W=W, KC=2, R=8):
    HO, WO = H // 2, W // 2
    A = R // 2               # output rows per partition
    PB = H // R              # partitions per channel (64)
    assert KC * PB == 128
    n_blocks = C // KC
    FD = R * W               # free-dim elems per partition (4096)

    nc = bacc.Bacc("TRN2", target_bir_lowering=False, debug=False)
    x = nc.dram_tensor("x", [C, H, W], F32, kind="ExternalInput").ap()
    out = nc.dram_tensor("out", [4 * C, HO, WO], F32, kind="ExternalOutput").ap()

    with tile.TileContext(nc) as tc, ExitStack() as ctx:
        xpool = ctx.enter_context(tc.tile_pool(name="xp", bufs=5))
        mpool = ctx.enter_context(tc.tile_pool(name="mp", bufs=2))
        rpool = ctx.enter_context(tc.tile_pool(name="rp", bufs=4))

        xts = {}

        def emit_load(i):
            c0 = i * KC
            xt = xpool.tile([128, FD], F32)
            src = x[c0 : c0 + KC].rearrange("k (b f) w -> (k b) f w", f=R)
            nc.scalar.dma_start(xt[:].rearrange("p (f w) -> p f w", w=W), src)
            xts[i] = xt

        def emit_compute_store(i):
            c0 = i * KC
            xt = xts.pop(i)

            # ---- halve in place on ACT (activation Copy, scale 0.5)
            nc.scalar.mul(xt[:], xt[:], 0.5)

            x4 = xt[:].rearrange("p (a t w) -> p a t w", t=2, w=W)
            top, bot = x4[:, :, 0, :], x4[:, :, 1, :]

            # ---- vertical butterfly (DVE), s/d interleaved in one tile
            m_t = mpool.tile([128, 2 * A * W], F32)
            mv = m_t[:].rearrange("p (v a w) -> p v a w", v=2, a=A)
            nc.vector.tensor_tensor(mv[:, 0], top, bot, ADD)   # s
            nc.vector.tensor_tensor(mv[:, 1], bot, top, SUB)   # d

            # ---- horizontal butterfly (DVE), 2 fused ops
            m5 = m_t[:].rearrange("p (v a j t) -> p v a j t", v=2, a=A, t=2)
            ev, od = m5[:, :, :, :, 0], m5[:, :, :, :, 1]
            rt = rpool.tile([128, 4 * A * WO], F32)
            r4 = rt[:].rearrange("p (u a j) -> p u a j", u=4, a=A)
            nc.vector.tensor_tensor(r4[:, 0:2], ev, od, ADD)   # ll, lh
            nc.vector.tensor_tensor(r4[:, 2:4], od, ev, SUB)   # hl, hh

            # ---- store: 4 runs of A*WO*4 = 4 KB contiguous per partition.
            # DMA APs are capped at 3 dims, so one DMA per channel:
            # dst (b, q, r*w), src = partition slice (b, q, r*w).
            for k in range(KC):
                ck = c0 + k
                dst = out[4 * ck : 4 * ck + 4].rearrange(
                    "q (b r) w -> b q (r w)", r=A
                )
                src = rt[k * PB : (k + 1) * PB].rearrange(
                    "b (q f) -> b q f", q=4
                )
                nc.sync.dma_start(dst, src)

        PF = 4  # load prefetch depth (requires xpool bufs >= PF + 1)
        for i in range(PF):
            emit_load(i)
        for i in range(n_blocks):
            if i + PF < n_blocks:
                emit_load(i + PF)
            emit_compute_store(i)
    nc.compile()
    return nc


def _get_nc():
    if "nc" not in _CACHED:
        _CACHED["nc"] = _build()
    return _CACHED["nc"]


def _run(x, **kwargs):
    x = np.ascontiguousarray(np.asarray(x), dtype=np.float32)
    assert x.shape == (N_CORES, C, H, W), x.shape
    nc = _get_nc()
    in_maps = [{"x": np.ascontiguousarray(x[i])} for i in range(N_CORES)]
    res = run_bass_kernel_spmd(nc, in_maps, core_ids=list(range(N_CORES)), **kwargs)
    out = np.stack([res.results[i]["out"] for i in range(N_CORES)], axis=0)
    return out, res


def kernel(x):
    return _run(x)[0]
